# revision 7
# baseline (speedup 1.0000x reference)
"""Bahdanau-attention kernel for Trainium2, data-parallel over batch on 8 NeuronCores.

Per-core shard: 8 batches. Pipeline per core:
  1. Setup: cast-DMA W2/W1/dec to bf16, PE-transpose to get E on partitions,
     h_proj = dec @ W1.T in f32 PSUM, fold W1_b+W2_b into per-partition bias
     columns Hb.
  2. Main loop over flat (b*p) row chunks [384,384,384,416]:
     cast-DMA enc rows (f32->bf16) -> PE-transpose to encT -> bf16 matmuls
     accumulate enc_projT in f32 PSUM -> ScalarE tanh with per-partition bias
     Hb -> V-dot matmul accumulates scores.
  3. Per completed batch: softmax (f32) on its scores row, attn out, w -> wT
     transpose, context matmul (wT stationary, re-DMA'd bf16 enc rows moving).
Matmul operands are bf16 (1 cyc/row on PE); all accumulation is f32.
"""

import sys

sys.path.insert(0, "/opt/trn_rl_repo")
sys.path.insert(0, "/opt/pypackages")

import numpy as np
from contextlib import ExitStack

import concourse.bass as bass
import concourse.bacc as bacc
import concourse.mybir as mybir
import concourse.tile as tile
from concourse.masks import make_identity

F32 = mybir.dt.float32
BF16 = mybir.dt.bfloat16
AF = mybir.ActivationFunctionType
ALU = mybir.AluOpType
AX = mybir.AxisListType

B, P, E, A = 64, 196, 2048, 1024
NCORES = 8
BL = B // NCORES          # 8 local batches per core
BP = BL * P               # 1568 flat rows per core
KE = E // 128             # 16 contraction chunks
KA = A // 128             # 8 attn-dim tiles
CHUNKS = [(0, 384), (384, 384), (768, 384), (1152, 416)]
CW = 416                  # max chunk width


def _segments(S, W):
    """Batch segments [(b, off_in_chunk, len)] covering chunk rows [S, S+W)."""
    segs = []
    b = S // P
    pos = S
    while pos < S + W:
        end = min((b + 1) * P, S + W)
        segs.append((b, pos - S, end - pos))
        pos = end
        b += 1
    return segs


def _done_batches():
    done = []
    prev = 0
    for S, W in CHUNKS:
        nb = (S + W) // P
        done.append(list(range(prev, nb)))
        prev = nb
    return done


DONE = _done_batches()


def build_nc():
    nc = bacc.Bacc("TRN2", target_bir_lowering=False, debug=False,
                   enable_asserts=False)

    enc = nc.dram_tensor("enc_hiddens", [BL, P, E], F32, kind="ExternalInput")
    dec = nc.dram_tensor("dec_prev_hidden", [BL, E], F32, kind="ExternalInput")
    w1 = nc.dram_tensor("W1_w", [A, E], F32, kind="ExternalInput")
    w1b = nc.dram_tensor("W1_b", [A], F32, kind="ExternalInput")
    w2 = nc.dram_tensor("W2_w", [A, E], F32, kind="ExternalInput")
    w2b = nc.dram_tensor("W2_b", [A], F32, kind="ExternalInput")
    vw = nc.dram_tensor("V_w", [1, A], F32, kind="ExternalInput")
    out_ctx = nc.dram_tensor("out_ctx", [BL, E], F32, kind="ExternalOutput")
    out_attn = nc.dram_tensor("out_attn", [BL, P], F32, kind="ExternalOutput")

    enc_flat = enc.ap().rearrange("b p e -> (b p) e")

    with tile.TileContext(nc) as tc:
        with ExitStack() as ctx:
            const = ctx.enter_context(tc.tile_pool(name="const", bufs=1))
            identb = const.tile([128, 128], BF16)
            make_identity(nc, identb[:])
            identf = const.tile([128, 128], F32)
            make_identity(nc, identf[:])

            # Persistent SBUF tensors
            w2t = const.tile([128, KE * A], BF16)       # W2^T: [e, k-major a]
            hb = const.tile([128, KA * BL], F32)        # Hb[q, a*BL+b]
            vt = const.tile([128, KA], BF16)            # V^T columns per a-tile
            scores = const.tile([1, BP], F32)

            tp_ps = ctx.enter_context(
                tc.tile_pool(name="tp_ps", bufs=2, space="PSUM"))

            # ---------------- setup ----------------
            with ExitStack() as sctx:
                spool = sctx.enter_context(tc.tile_pool(name="setup", bufs=2))
                sp1 = sctx.enter_context(tc.tile_pool(name="setup1", bufs=1))
                hps_pool = sctx.enter_context(
                    tc.tile_pool(name="hps", bufs=2, space="PSUM"))

                # W2 -> w2t (cast-DMA to bf16, then SBUF->SBUF xbar DMA transpose)
                for a in range(KA):
                    wnat = spool.tile([128, E], BF16, tag="wnat")
                    nc.gpsimd.dma_start(wnat[:], w2.ap()[a * 128:(a + 1) * 128, :])
                    for k in range(KE):
                        nc.sync.dma_start_transpose(
                            w2t[:, k * A + a * 128: k * A + (a + 1) * 128],
                            wnat[:, k * 128:(k + 1) * 128])

                # W1 -> w1t (scratch, freed after setup)
                w1t = sp1.tile([128, KE * A], BF16)
                for a in range(KA):
                    wnat = spool.tile([128, E], BF16, tag="wnat")
                    nc.gpsimd.dma_start(wnat[:], w1.ap()[a * 128:(a + 1) * 128, :])
                    for k in range(KE):
                        nc.sync.dma_start_transpose(
                            w1t[:, k * A + a * 128: k * A + (a + 1) * 128],
                            wnat[:, k * 128:(k + 1) * 128])

                # dec -> decT (bf16)
                dec_sb = sp1.tile([BL, E], BF16)
                nc.gpsimd.dma_start(dec_sb[:], dec.ap())
                dect = sp1.tile([128, KE * BL], BF16)
                for k in range(KE):
                    ps = tp_ps.tile([128, 128], BF16, tag="tp")
                    nc.tensor.transpose(
                        ps[:, :BL], dec_sb[:, k * 128:(k + 1) * 128],
                        identb[:BL, :BL])
                    nc.any.tensor_copy(dect[:, k * BL:(k + 1) * BL], ps[:, :BL])

                # h_proj = dec @ W1.T  -> [BL, A] f32
                h_sb = sp1.tile([BL, A], F32)
                for half in range(2):
                    hps = hps_pool.tile([BL, 512], F32, tag="hps")
                    for k in range(KE):
                        nc.tensor.matmul(
                            hps[:],
                            dect[:, k * BL:(k + 1) * BL],
                            w1t[:, k * A + half * 512: k * A + half * 512 + 512],
                            start=(k == 0), stop=(k == KE - 1))
                    nc.any.tensor_copy(h_sb[:, half * 512:(half + 1) * 512], hps[:])

                # bias columns: W1_b + W2_b, laid out [q, a]  (f32)
                w1bc = sp1.tile([128, KA], F32)
                w2bc = sp1.tile([128, KA], F32)
                with nc.allow_non_contiguous_dma(reason="tiny bias transpose loads"):
                    nc.sync.dma_start(w1bc[:], w1b.ap().rearrange("(a k) -> k a", k=128))
                    nc.sync.dma_start(w2bc[:], w2b.ap().rearrange("(a k) -> k a", k=128))
                    nc.gpsimd.dma_start(vt[:], vw.ap().rearrange("o (a k) -> k (o a)", k=128))
                nc.vector.tensor_add(w1bc[:], w1bc[:], w2bc[:])

                # hT + bias -> Hb  (f32 transpose of h_sb)
                for a in range(KA):
                    ps = hps_pool.tile([128, 128], F32, tag="tpf")
                    nc.tensor.transpose(
                        ps[:, :BL], h_sb[:, a * 128:(a + 1) * 128],
                        identf[:BL, :BL])
                    nc.vector.tensor_scalar(
                        out=hb[:, a * BL:(a + 1) * BL], in0=ps[:, :BL],
                        scalar1=w1bc[:, a:a + 1], scalar2=None, op0=ALU.add)

            # ---------------- main loop ----------------
            dramp = ctx.enter_context(tc.tile_pool(name="dram", bufs=1, space="DRAM"))
            stage = dramp.tile([BP, E], BF16)   # bf16 copy of enc rows
            enctp = ctx.enter_context(tc.tile_pool(name="enct", bufs=2))
            tpool = ctx.enter_context(tc.tile_pool(name="tpool", bufs=3))
            ebp = ctx.enter_context(tc.tile_pool(name="eb", bufs=4))
            ctxp = ctx.enter_context(tc.tile_pool(name="ctxsb", bufs=2))
            smp = ctx.enter_context(tc.tile_pool(name="smp", bufs=2))
            bigps = ctx.enter_context(tc.tile_pool(name="bigps", bufs=3, space="PSUM"))
            scps = ctx.enter_context(tc.tile_pool(name="scps", bufs=1, space="PSUM"))
            ctxps = ctx.enter_context(tc.tile_pool(name="ctxps", bufs=1, space="PSUM"))

            for ci, (S, W) in enumerate(CHUNKS):
                # cast this chunk's rows to bf16 in DRAM, then DMA-transpose in
                nc.gpsimd.dma_start(stage[S:S + W, :], enc_flat[S:S + W, :])
                enct = enctp.tile([128, KE * CW], BF16, tag="enct")
                for k in range(KE):
                    nc.sync.dma_start_transpose(
                        enct[:, k * CW: k * CW + W],
                        stage[S:S + W, k * 128:(k + 1) * 128])

                segs = _segments(S, W)
                sc = scps.tile([1, CW], F32, tag="sc")
                for a in range(KA):
                    ps = bigps.tile([128, CW], F32, tag="big")
                    for k in range(KE):
                        nc.tensor.matmul(
                            ps[:, :W],
                            w2t[:, k * A + a * 128: k * A + (a + 1) * 128],
                            enct[:, k * CW: k * CW + W],
                            start=(k == 0), stop=(k == KE - 1))
                    t_sb = tpool.tile([128, CW], BF16, tag="t")
                    for (b, off, ln) in segs:
                        nc.scalar.activation(
                            t_sb[:, off:off + ln], ps[:, off:off + ln], AF.Tanh,
                            bias=hb[:, a * BL + b: a * BL + b + 1])
                    nc.tensor.matmul(
                        sc[:, :W], vt[:, a:a + 1], t_sb[:, :W],
                        start=(a == 0), stop=(a == KA - 1))
                nc.any.tensor_copy(scores[:, S:S + W], sc[:, :W])

                for b in DONE[ci]:
                    sseg = scores[:, b * P:(b + 1) * P]
                    negmax = smp.tile([1, 1], F32, tag="nm")
                    nc.vector.reduce_max(negmax[:], sseg, axis=AX.X, negate=True)
                    ex = smp.tile([1, P], F32, tag="ex")
                    nc.scalar.activation(ex[:], sseg, AF.Exp, bias=negmax[:])
                    ssum = smp.tile([1, 1], F32, tag="sm")
                    nc.vector.reduce_sum(ssum[:], ex[:], axis=AX.X)
                    rcp = smp.tile([1, 1], F32, tag="rc")
                    nc.vector.reciprocal(rcp[:], ssum[:])
                    wsb = smp.tile([1, P], F32, tag="w")
                    nc.vector.tensor_scalar(
                        out=wsb[:], in0=ex[:], scalar1=rcp[:], scalar2=None,
                        op0=ALU.mult)
                    nc.sync.dma_start(out_attn.ap()[b:b + 1, :], wsb[:])

                    # w -> wT (bf16, two partition chunks: 128 + 68)
                    wbf = smp.tile([1, P], BF16, tag="wbf")
                    nc.vector.tensor_copy(wbf[:], wsb[:])
                    wt = smp.tile([128, 2], BF16, tag="wt")
                    ps = tp_ps.tile([128, 128], BF16, tag="tp")
                    nc.tensor.transpose(ps[:, :1], wbf[:, :128], identb[:1, :1])
                    nc.vector.tensor_copy(wt[:, 0:1], ps[:, :1])
                    ps = tp_ps.tile([128, 128], BF16, tag="tp")
                    nc.tensor.transpose(ps[:P - 128, :1], wbf[:, 128:P],
                                        identb[:1, :1])
                    nc.vector.tensor_copy(wt[:P - 128, 1:2], ps[:P - 128, :1])

                    # context = w @ enc_b  (re-DMA bf16 rows from stage)
                    eb0 = ebp.tile([128, E], BF16, tag="eb")
                    eb1 = ebp.tile([128, E], BF16, tag="eb")
                    nc.sync.dma_start(eb0[:], stage[b * P: b * P + 128, :])
                    nc.sync.dma_start(eb1[:P - 128, :],
                                      stage[b * P + 128:(b + 1) * P, :])
                    ctx_sb = ctxp.tile([1, E], F32, tag="ctx")
                    for ec in range(4):
                        cps = ctxps.tile([1, 512], F32, tag="cps")
                        nc.tensor.matmul(
                            cps[:], wt[:, 0:1],
                            eb0[:, ec * 512:(ec + 1) * 512],
                            start=True, stop=False)
                        nc.tensor.matmul(
                            cps[:], wt[:P - 128, 1:2],
                            eb1[:P - 128, ec * 512:(ec + 1) * 512],
                            start=False, stop=True)
                        nc.any.tensor_copy(ctx_sb[:, ec * 512:(ec + 1) * 512],
                                           cps[:])
                    nc.sync.dma_start(out_ctx.ap()[b:b + 1, :], ctx_sb[:])

    nc.compile()
    return nc


_NC = None


def _get_nc():
    global _NC
    if _NC is None:
        _NC = build_nc()
    return _NC


def kernel(enc_hiddens, dec_prev_hidden, W1_w, W1_b, W2_w, W2_b, V_w, V_b):
    from concourse import bass_utils

    nc = _get_nc()
    enc_hiddens = np.asarray(enc_hiddens, np.float32)
    dec_prev_hidden = np.asarray(dec_prev_hidden, np.float32)
    shared = {
        "W1_w": np.ascontiguousarray(W1_w, np.float32),
        "W1_b": np.ascontiguousarray(W1_b, np.float32),
        "W2_w": np.ascontiguousarray(W2_w, np.float32),
        "W2_b": np.ascontiguousarray(W2_b, np.float32),
        "V_w": np.ascontiguousarray(V_w, np.float32),
    }
    in_maps = []
    for i in range(NCORES):
        m = dict(shared)
        m["enc_hiddens"] = np.ascontiguousarray(enc_hiddens[i * BL:(i + 1) * BL])
        m["dec_prev_hidden"] = np.ascontiguousarray(
            dec_prev_hidden[i * BL:(i + 1) * BL])
        in_maps.append(m)

    res = bass_utils.run_bass_kernel_spmd(nc, in_maps, core_ids=list(range(NCORES)))
    outs = res.results
    context = np.concatenate([o["out_ctx"] for o in outs], axis=0).reshape(B, 1, E)
    attn = np.concatenate([o["out_attn"] for o in outs], axis=0)
    return context, attn


# revision 10
# speedup vs baseline: 1.7874x; 1.7874x over previous
"""Bahdanau-attention kernel for Trainium2, data-parallel over batch on 8 NeuronCores.

Per-core shard: 8 batches. Pipeline per core:
  1. Setup: cast-DMA W2/W1/dec to bf16, PE-transpose to get E on partitions,
     h_proj = dec @ W1.T in f32 PSUM, fold W1_b+W2_b into per-partition bias
     columns Hb.
  2. Main loop over flat (b*p) row chunks [384,384,384,416]:
     cast-DMA enc rows (f32->bf16) -> PE-transpose to encT -> bf16 matmuls
     accumulate enc_projT in f32 PSUM -> ScalarE tanh with per-partition bias
     Hb -> V-dot matmul accumulates scores.
  3. Per completed batch: softmax (f32) on its scores row, attn out, w -> wT
     transpose, context matmul (wT stationary, re-DMA'd bf16 enc rows moving).
Matmul operands are bf16 (1 cyc/row on PE); all accumulation is f32.
"""

import sys

sys.path.insert(0, "/opt/trn_rl_repo")
sys.path.insert(0, "/opt/pypackages")

import numpy as np
from contextlib import ExitStack

import concourse.bass as bass
import concourse.bacc as bacc
import concourse.mybir as mybir
import concourse.tile as tile
from concourse.masks import make_identity

F32 = mybir.dt.float32
BF16 = mybir.dt.bfloat16
AF = mybir.ActivationFunctionType
ALU = mybir.AluOpType
AX = mybir.AxisListType

B, P, E, A = 64, 196, 2048, 1024
NCORES = 8
BL = B // NCORES          # 8 local batches per core
BP = BL * P               # 1568 flat rows per core
KE = E // 128             # 16 contraction chunks
KA = A // 128             # 8 attn-dim tiles
CHUNKS = [(0, 384), (384, 384), (768, 384), (1152, 416)]
CW = 416                  # max chunk width


def _segments(S, W):
    """Batch segments [(b, off_in_chunk, len)] covering chunk rows [S, S+W)."""
    segs = []
    b = S // P
    pos = S
    while pos < S + W:
        end = min((b + 1) * P, S + W)
        segs.append((b, pos - S, end - pos))
        pos = end
        b += 1
    return segs


def _done_batches():
    done = []
    prev = 0
    for S, W in CHUNKS:
        nb = (S + W) // P
        done.append(list(range(prev, nb)))
        prev = nb
    return done


DONE = _done_batches()


def build_nc():
    nc = bacc.Bacc("TRN2", target_bir_lowering=False, debug=False,
                   enable_asserts=False)

    enc = nc.dram_tensor("enc_hiddens", [BL, P, E], F32, kind="ExternalInput")
    dec = nc.dram_tensor("dec_prev_hidden", [BL, E], F32, kind="ExternalInput")
    w1 = nc.dram_tensor("W1_w", [A, E], F32, kind="ExternalInput")
    w1b = nc.dram_tensor("W1_b", [A], F32, kind="ExternalInput")
    w2 = nc.dram_tensor("W2_w", [A, E], F32, kind="ExternalInput")
    w2b = nc.dram_tensor("W2_b", [A], F32, kind="ExternalInput")
    vw = nc.dram_tensor("V_w", [1, A], F32, kind="ExternalInput")
    out_ctx = nc.dram_tensor("out_ctx", [BL, E], F32, kind="ExternalOutput")
    out_attn = nc.dram_tensor("out_attn", [BL, P], F32, kind="ExternalOutput")

    enc_flat = enc.ap().rearrange("b p e -> (b p) e")

    with tile.TileContext(nc) as tc:
        with ExitStack() as ctx:
            const = ctx.enter_context(tc.tile_pool(name="const", bufs=1))
            identb = const.tile([128, 128], BF16)
            make_identity(nc, identb[:])
            identf = const.tile([128, 128], F32)
            make_identity(nc, identf[:])

            # Persistent SBUF tensors
            w2t = const.tile([128, KE * A], BF16)       # W2^T: [e, k-major a]
            hb = const.tile([128, KA * BL], F32)        # Hb[q, a*BL+b]
            vt = const.tile([128, KA], BF16)            # V^T columns per a-tile
            scores = const.tile([1, BP], F32)

            tp_ps = ctx.enter_context(
                tc.tile_pool(name="tp_ps", bufs=2, space="PSUM"))

            dramp = ctx.enter_context(tc.tile_pool(name="dram", bufs=1, space="DRAM"))
            stage = dramp.tile([BP, E], BF16)    # bf16 copy of enc rows
            w2stage = dramp.tile([A, E], BF16)
            w1stage = dramp.tile([A, E], BF16)

            # Cast-DMA order: enc chunk0 first (unblocks first matmuls), then
            # weights (k-sliced so transposed reads can start early), then the
            # remaining enc chunks.
            S0, W0 = CHUNKS[0]
            nc.gpsimd.dma_start(stage[S0:S0 + W0, :], enc_flat[S0:S0 + W0, :])
            for q in range(4):
                nc.gpsimd.dma_start(w2stage[:, q * 512:(q + 1) * 512],
                                    w2.ap()[:, q * 512:(q + 1) * 512])
            for q in range(4):
                nc.gpsimd.dma_start(w1stage[:, q * 512:(q + 1) * 512],
                                    w1.ap()[:, q * 512:(q + 1) * 512])
            for ci in range(1, len(CHUNKS)):
                S, W = CHUNKS[ci]
                nc.gpsimd.dma_start(stage[S:S + W, :], enc_flat[S:S + W, :])

            # ---------------- setup ----------------
            with ExitStack() as sctx:
                sp1 = sctx.enter_context(tc.tile_pool(name="setup1", bufs=1))
                hps_pool = sctx.enter_context(
                    tc.tile_pool(name="hps", bufs=2, space="PSUM"))

                # W2T / W1T via DRAM->SBUF xbar DMA transpose
                for k in range(KE):
                    nc.sync.dma_start_transpose(
                        w2t[:, k * A:(k + 1) * A],
                        w2stage[:, k * 128:(k + 1) * 128])
                w1t = sp1.tile([128, KE * A], BF16)
                for k in range(KE):
                    nc.sync.dma_start_transpose(
                        w1t[:, k * A:(k + 1) * A],
                        w1stage[:, k * 128:(k + 1) * 128])

                # dec -> decT (bf16)
                dec_sb = sp1.tile([BL, E], BF16)
                nc.gpsimd.dma_start(dec_sb[:], dec.ap())
                dect = sp1.tile([128, KE * BL], BF16)
                for k in range(KE):
                    ps = tp_ps.tile([128, 128], BF16, tag="tp")
                    nc.tensor.transpose(
                        ps[:, :BL], dec_sb[:, k * 128:(k + 1) * 128],
                        identb[:BL, :BL])
                    nc.any.tensor_copy(dect[:, k * BL:(k + 1) * BL], ps[:, :BL])

                # h_proj = dec @ W1.T  -> [BL, A] f32
                h_sb = sp1.tile([BL, A], F32)
                for half in range(2):
                    hps = hps_pool.tile([BL, 512], F32, tag="hps")
                    for k in range(KE):
                        nc.tensor.matmul(
                            hps[:],
                            dect[:, k * BL:(k + 1) * BL],
                            w1t[:, k * A + half * 512: k * A + half * 512 + 512],
                            start=(k == 0), stop=(k == KE - 1))
                    nc.any.tensor_copy(h_sb[:, half * 512:(half + 1) * 512], hps[:])

                # bias columns: W1_b + W2_b, laid out [q, a]  (f32)
                w1bc = sp1.tile([128, KA], F32)
                w2bc = sp1.tile([128, KA], F32)
                with nc.allow_non_contiguous_dma(reason="tiny bias transpose loads"):
                    nc.sync.dma_start(w1bc[:], w1b.ap().rearrange("(a k) -> k a", k=128))
                    nc.sync.dma_start(w2bc[:], w2b.ap().rearrange("(a k) -> k a", k=128))
                    nc.gpsimd.dma_start(vt[:], vw.ap().rearrange("o (a k) -> k (o a)", k=128))
                nc.vector.tensor_add(w1bc[:], w1bc[:], w2bc[:])

                # hT + bias -> Hb  (f32 transpose of h_sb)
                for a in range(KA):
                    ps = hps_pool.tile([128, 128], F32, tag="tpf")
                    nc.tensor.transpose(
                        ps[:, :BL], h_sb[:, a * 128:(a + 1) * 128],
                        identf[:BL, :BL])
                    nc.vector.tensor_scalar(
                        out=hb[:, a * BL:(a + 1) * BL], in0=ps[:, :BL],
                        scalar1=w1bc[:, a:a + 1], scalar2=None, op0=ALU.add)

            # ---------------- main loop ----------------
            enctp = ctx.enter_context(tc.tile_pool(name="enct", bufs=2))
            tpool = ctx.enter_context(tc.tile_pool(name="tpool", bufs=3))
            ebp = ctx.enter_context(tc.tile_pool(name="eb", bufs=4))
            ctxp = ctx.enter_context(tc.tile_pool(name="ctxsb", bufs=2))
            smp = ctx.enter_context(tc.tile_pool(name="smp", bufs=2))
            bigps = ctx.enter_context(tc.tile_pool(name="bigps", bufs=3, space="PSUM"))
            scps = ctx.enter_context(tc.tile_pool(name="scps", bufs=1, space="PSUM"))
            ctxps = ctx.enter_context(tc.tile_pool(name="ctxps", bufs=1, space="PSUM"))

            for ci, (S, W) in enumerate(CHUNKS):
                enct = enctp.tile([128, KE * CW], BF16, tag="enct")
                for k in range(KE):
                    nc.sync.dma_start_transpose(
                        enct[:, k * CW: k * CW + W],
                        stage[S:S + W, k * 128:(k + 1) * 128])

                segs = _segments(S, W)
                sc = scps.tile([1, CW], F32, tag="sc")
                for a in range(KA):
                    ps = bigps.tile([128, CW], F32, tag="big")
                    for k in range(KE):
                        nc.tensor.matmul(
                            ps[:, :W],
                            w2t[:, k * A + a * 128: k * A + (a + 1) * 128],
                            enct[:, k * CW: k * CW + W],
                            start=(k == 0), stop=(k == KE - 1))
                    t_sb = tpool.tile([128, CW], BF16, tag="t")
                    for (b, off, ln) in segs:
                        nc.scalar.activation(
                            t_sb[:, off:off + ln], ps[:, off:off + ln], AF.Tanh,
                            bias=hb[:, a * BL + b: a * BL + b + 1])
                    nc.tensor.matmul(
                        sc[:, :W], vt[:, a:a + 1], t_sb[:, :W],
                        start=(a == 0), stop=(a == KA - 1))
                nc.any.tensor_copy(scores[:, S:S + W], sc[:, :W])

                for b in DONE[ci]:
                    sseg = scores[:, b * P:(b + 1) * P]
                    negmax = smp.tile([1, 1], F32, tag="nm")
                    nc.vector.reduce_max(negmax[:], sseg, axis=AX.X, negate=True)
                    ex = smp.tile([1, P], F32, tag="ex")
                    nc.scalar.activation(ex[:], sseg, AF.Exp, bias=negmax[:])
                    ssum = smp.tile([1, 1], F32, tag="sm")
                    nc.vector.reduce_sum(ssum[:], ex[:], axis=AX.X)
                    rcp = smp.tile([1, 1], F32, tag="rc")
                    nc.vector.reciprocal(rcp[:], ssum[:])
                    wsb = smp.tile([1, P], F32, tag="w")
                    nc.vector.tensor_scalar(
                        out=wsb[:], in0=ex[:], scalar1=rcp[:], scalar2=None,
                        op0=ALU.mult)
                    nc.sync.dma_start(out_attn.ap()[b:b + 1, :], wsb[:])

                    # w -> wT (bf16, two partition chunks: 128 + 68)
                    wbf = smp.tile([1, P], BF16, tag="wbf")
                    nc.vector.tensor_copy(wbf[:], wsb[:])
                    wt = smp.tile([128, 2], BF16, tag="wt")
                    ps = tp_ps.tile([128, 128], BF16, tag="tp")
                    nc.tensor.transpose(ps[:, :1], wbf[:, :128], identb[:1, :1])
                    nc.vector.tensor_copy(wt[:, 0:1], ps[:, :1])
                    ps = tp_ps.tile([128, 128], BF16, tag="tp")
                    nc.tensor.transpose(ps[:P - 128, :1], wbf[:, 128:P],
                                        identb[:1, :1])
                    nc.vector.tensor_copy(wt[:P - 128, 1:2], ps[:P - 128, :1])

                    # context = w @ enc_b  (re-DMA bf16 rows from stage)
                    eb0 = ebp.tile([128, E], BF16, tag="eb")
                    eb1 = ebp.tile([128, E], BF16, tag="eb")
                    nc.sync.dma_start(eb0[:], stage[b * P: b * P + 128, :])
                    nc.sync.dma_start(eb1[:P - 128, :],
                                      stage[b * P + 128:(b + 1) * P, :])
                    ctx_sb = ctxp.tile([1, E], F32, tag="ctx")
                    for ec in range(4):
                        cps = ctxps.tile([1, 512], F32, tag="cps")
                        nc.tensor.matmul(
                            cps[:], wt[:, 0:1],
                            eb0[:, ec * 512:(ec + 1) * 512],
                            start=True, stop=False)
                        nc.tensor.matmul(
                            cps[:], wt[:P - 128, 1:2],
                            eb1[:P - 128, ec * 512:(ec + 1) * 512],
                            start=False, stop=True)
                        nc.any.tensor_copy(ctx_sb[:, ec * 512:(ec + 1) * 512],
                                           cps[:])
                    nc.sync.dma_start(out_ctx.ap()[b:b + 1, :], ctx_sb[:])

    nc.compile()
    return nc


_NC = None


def _get_nc():
    global _NC
    if _NC is None:
        _NC = build_nc()
    return _NC


def kernel(enc_hiddens, dec_prev_hidden, W1_w, W1_b, W2_w, W2_b, V_w, V_b):
    from concourse import bass_utils

    nc = _get_nc()
    enc_hiddens = np.asarray(enc_hiddens, np.float32)
    dec_prev_hidden = np.asarray(dec_prev_hidden, np.float32)
    shared = {
        "W1_w": np.ascontiguousarray(W1_w, np.float32),
        "W1_b": np.ascontiguousarray(W1_b, np.float32),
        "W2_w": np.ascontiguousarray(W2_w, np.float32),
        "W2_b": np.ascontiguousarray(W2_b, np.float32),
        "V_w": np.ascontiguousarray(V_w, np.float32),
    }
    in_maps = []
    for i in range(NCORES):
        m = dict(shared)
        m["enc_hiddens"] = np.ascontiguousarray(enc_hiddens[i * BL:(i + 1) * BL])
        m["dec_prev_hidden"] = np.ascontiguousarray(
            dec_prev_hidden[i * BL:(i + 1) * BL])
        in_maps.append(m)

    res = bass_utils.run_bass_kernel_spmd(nc, in_maps, core_ids=list(range(NCORES)))
    outs = res.results
    context = np.concatenate([o["out_ctx"] for o in outs], axis=0).reshape(B, 1, E)
    attn = np.concatenate([o["out_attn"] for o in outs], axis=0)
    return context, attn


# revision 12
# speedup vs baseline: 1.9234x; 1.0761x over previous
"""Bahdanau-attention kernel for Trainium2, data-parallel over batch on 8 NeuronCores.

Per-core shard: 8 batches. Pipeline per core:
  1. Setup: cast-DMA W2/W1/dec to bf16, PE-transpose to get E on partitions,
     h_proj = dec @ W1.T in f32 PSUM, fold W1_b+W2_b into per-partition bias
     columns Hb.
  2. Main loop over flat (b*p) row chunks [384,384,384,416]:
     cast-DMA enc rows (f32->bf16) -> PE-transpose to encT -> bf16 matmuls
     accumulate enc_projT in f32 PSUM -> ScalarE tanh with per-partition bias
     Hb -> V-dot matmul accumulates scores.
  3. Per completed batch: softmax (f32) on its scores row, attn out, w -> wT
     transpose, context matmul (wT stationary, re-DMA'd bf16 enc rows moving).
Matmul operands are bf16 (1 cyc/row on PE); all accumulation is f32.
"""

import sys

sys.path.insert(0, "/opt/trn_rl_repo")
sys.path.insert(0, "/opt/pypackages")

import numpy as np
from contextlib import ExitStack

import concourse.bass as bass
import concourse.bacc as bacc
import concourse.mybir as mybir
import concourse.tile as tile
from concourse.masks import make_identity

F32 = mybir.dt.float32
BF16 = mybir.dt.bfloat16
AF = mybir.ActivationFunctionType
ALU = mybir.AluOpType
AX = mybir.AxisListType

B, P, E, A = 64, 196, 2048, 1024
NCORES = 8
BL = B // NCORES          # 8 local batches per core
BP = BL * P               # 1568 flat rows per core
KE = E // 128             # 16 contraction chunks
KA = A // 128             # 8 attn-dim tiles
CHUNKS = [(0, 384), (384, 384), (768, 384), (1152, 416)]
CW = 416                  # max chunk width


def _segments(S, W):
    """Batch segments [(b, off_in_chunk, len)] covering chunk rows [S, S+W)."""
    segs = []
    b = S // P
    pos = S
    while pos < S + W:
        end = min((b + 1) * P, S + W)
        segs.append((b, pos - S, end - pos))
        pos = end
        b += 1
    return segs


def _done_batches():
    done = []
    prev = 0
    for S, W in CHUNKS:
        nb = (S + W) // P
        done.append(list(range(prev, nb)))
        prev = nb
    return done


DONE = _done_batches()


def build_nc():
    nc = bacc.Bacc("TRN2", target_bir_lowering=False, debug=False,
                   enable_asserts=False)

    enc = nc.dram_tensor("enc_hiddens", [BL, P, E], F32, kind="ExternalInput")
    dec = nc.dram_tensor("dec_prev_hidden", [BL, E], F32, kind="ExternalInput")
    w1 = nc.dram_tensor("W1_w", [A, E], F32, kind="ExternalInput")
    w1b = nc.dram_tensor("W1_b", [A], F32, kind="ExternalInput")
    w2 = nc.dram_tensor("W2_w", [A, E], F32, kind="ExternalInput")
    w2b = nc.dram_tensor("W2_b", [A], F32, kind="ExternalInput")
    vw = nc.dram_tensor("V_w", [1, A], F32, kind="ExternalInput")
    out_ctx = nc.dram_tensor("out_ctx", [BL, E], F32, kind="ExternalOutput")
    out_attn = nc.dram_tensor("out_attn", [BL, P], F32, kind="ExternalOutput")

    enc_flat = enc.ap().rearrange("b p e -> (b p) e")

    with tile.TileContext(nc) as tc:
        with ExitStack() as ctx:
            const = ctx.enter_context(tc.tile_pool(name="const", bufs=1))
            identb = const.tile([128, 128], BF16)
            make_identity(nc, identb[:])
            identf = const.tile([128, 128], F32)
            make_identity(nc, identf[:])

            # Persistent SBUF tensors
            w2t = const.tile([128, KE * A], BF16)       # W2^T: [e, k-major a]
            hb = const.tile([128, KA * BL], F32)        # Hb[q, a*BL+b]
            vt = const.tile([128, KA], BF16)            # V^T columns per a-tile
            scores = const.tile([1, BP], F32)

            tp_ps = ctx.enter_context(
                tc.tile_pool(name="tp_ps", bufs=2, space="PSUM"))

            dramp = ctx.enter_context(tc.tile_pool(name="dram", bufs=1, space="DRAM"))
            stage = dramp.tile([BP, E], BF16)    # bf16 copy of enc rows
            w2stage = dramp.tile([A, E], BF16)
            w1stage = dramp.tile([A, E], BF16)

            # Cast-DMA order: enc chunk0 first (unblocks first matmuls), then
            # weights (k-sliced so transposed reads can start early), then the
            # remaining enc chunks.
            S0, W0 = CHUNKS[0]
            nc.gpsimd.dma_start(stage[S0:S0 + W0, :], enc_flat[S0:S0 + W0, :])
            for q in range(4):
                nc.gpsimd.dma_start(w2stage[:, q * 512:(q + 1) * 512],
                                    w2.ap()[:, q * 512:(q + 1) * 512])
            for q in range(4):
                nc.gpsimd.dma_start(w1stage[:, q * 512:(q + 1) * 512],
                                    w1.ap()[:, q * 512:(q + 1) * 512])
            for ci in range(1, len(CHUNKS)):
                S, W = CHUNKS[ci]
                nc.gpsimd.dma_start(stage[S:S + W, :], enc_flat[S:S + W, :])

            # ---------------- setup ----------------
            with ExitStack() as sctx:
                sp1 = sctx.enter_context(tc.tile_pool(name="setup1", bufs=1))
                hps_pool = sctx.enter_context(
                    tc.tile_pool(name="hps", bufs=2, space="PSUM"))

                # W2T / W1T via DRAM->SBUF xbar DMA transpose. One instruction
                # per 512-col cast slice, 3D destination [128, 4k, A].
                w2t3 = w2t[:].rearrange("p (k a) -> p k a", k=KE)
                w1t = sp1.tile([128, KE * A], BF16)
                w1t3 = w1t[:].rearrange("p (k a) -> p k a", k=KE)
                for q in range(4):
                    nc.scalar.dma_start_transpose(
                        w2t3[:, 4 * q: 4 * (q + 1), :],
                        w2stage[:, q * 512:(q + 1) * 512])
                for q in range(4):
                    nc.scalar.dma_start_transpose(
                        w1t3[:, 4 * q: 4 * (q + 1), :],
                        w1stage[:, q * 512:(q + 1) * 512])

                # dec -> decT (bf16)
                dec_sb = sp1.tile([BL, E], BF16)
                nc.gpsimd.dma_start(dec_sb[:], dec.ap())
                dect = sp1.tile([128, KE * BL], BF16)
                for k in range(KE):
                    ps = tp_ps.tile([128, 128], BF16, tag="tp")
                    nc.tensor.transpose(
                        ps[:, :BL], dec_sb[:, k * 128:(k + 1) * 128],
                        identb[:BL, :BL])
                    nc.any.tensor_copy(dect[:, k * BL:(k + 1) * BL], ps[:, :BL])

                # h_proj = dec @ W1.T  -> [BL, A] f32
                h_sb = sp1.tile([BL, A], F32)
                for half in range(2):
                    hps = hps_pool.tile([BL, 512], F32, tag="hps")
                    for k in range(KE):
                        nc.tensor.matmul(
                            hps[:],
                            dect[:, k * BL:(k + 1) * BL],
                            w1t[:, k * A + half * 512: k * A + half * 512 + 512],
                            start=(k == 0), stop=(k == KE - 1))
                    nc.any.tensor_copy(h_sb[:, half * 512:(half + 1) * 512], hps[:])

                # bias columns: W1_b + W2_b, laid out [q, a]  (f32)
                w1bc = sp1.tile([128, KA], F32)
                w2bc = sp1.tile([128, KA], F32)
                with nc.allow_non_contiguous_dma(reason="tiny bias transpose loads"):
                    nc.sync.dma_start(w1bc[:], w1b.ap().rearrange("(a k) -> k a", k=128))
                    nc.sync.dma_start(w2bc[:], w2b.ap().rearrange("(a k) -> k a", k=128))
                    nc.gpsimd.dma_start(vt[:], vw.ap().rearrange("o (a k) -> k (o a)", k=128))
                nc.vector.tensor_add(w1bc[:], w1bc[:], w2bc[:])

                # hT + bias -> Hb  (f32 transpose of h_sb)
                for a in range(KA):
                    ps = hps_pool.tile([128, 128], F32, tag="tpf")
                    nc.tensor.transpose(
                        ps[:, :BL], h_sb[:, a * 128:(a + 1) * 128],
                        identf[:BL, :BL])
                    nc.vector.tensor_scalar(
                        out=hb[:, a * BL:(a + 1) * BL], in0=ps[:, :BL],
                        scalar1=w1bc[:, a:a + 1], scalar2=None, op0=ALU.add)

            # ---------------- main loop ----------------
            enctp = ctx.enter_context(tc.tile_pool(name="enct", bufs=2))
            tpool = ctx.enter_context(tc.tile_pool(name="tpool", bufs=3))
            ebp = ctx.enter_context(tc.tile_pool(name="eb", bufs=4))
            ctxp = ctx.enter_context(tc.tile_pool(name="ctxsb", bufs=2))
            smp = ctx.enter_context(tc.tile_pool(name="smp", bufs=2))
            bigps = ctx.enter_context(tc.tile_pool(name="bigps", bufs=3, space="PSUM"))
            scps = ctx.enter_context(tc.tile_pool(name="scps", bufs=1, space="PSUM"))
            ctxps = ctx.enter_context(tc.tile_pool(name="ctxps", bufs=1, space="PSUM"))

            for ci, (S, W) in enumerate(CHUNKS):
                enct = enctp.tile([128, KE * CW], BF16, tag="enct")
                enct3 = enct[:].rearrange("p (k w) -> p k w", w=CW)
                nc.sync.dma_start_transpose(
                    enct3[:, :, :W], stage[S:S + W, :])

                segs = _segments(S, W)
                sc = scps.tile([1, CW], F32, tag="sc")
                for a in range(KA):
                    ps = bigps.tile([128, CW], F32, tag="big")
                    for k in range(KE):
                        nc.tensor.matmul(
                            ps[:, :W],
                            w2t[:, k * A + a * 128: k * A + (a + 1) * 128],
                            enct[:, k * CW: k * CW + W],
                            start=(k == 0), stop=(k == KE - 1))
                    t_sb = tpool.tile([128, CW], BF16, tag="t")
                    for (b, off, ln) in segs:
                        nc.scalar.activation(
                            t_sb[:, off:off + ln], ps[:, off:off + ln], AF.Tanh,
                            bias=hb[:, a * BL + b: a * BL + b + 1])
                    nc.tensor.matmul(
                        sc[:, :W], vt[:, a:a + 1], t_sb[:, :W],
                        start=(a == 0), stop=(a == KA - 1))
                nc.any.tensor_copy(scores[:, S:S + W], sc[:, :W])

                for b in DONE[ci]:
                    sseg = scores[:, b * P:(b + 1) * P]
                    negmax = smp.tile([1, 1], F32, tag="nm")
                    nc.vector.reduce_max(negmax[:], sseg, axis=AX.X, negate=True)
                    ex = smp.tile([1, P], F32, tag="ex")
                    nc.scalar.activation(ex[:], sseg, AF.Exp, bias=negmax[:])
                    ssum = smp.tile([1, 1], F32, tag="sm")
                    nc.vector.reduce_sum(ssum[:], ex[:], axis=AX.X)
                    rcp = smp.tile([1, 1], F32, tag="rc")
                    nc.vector.reciprocal(rcp[:], ssum[:])
                    wsb = smp.tile([1, P], F32, tag="w")
                    nc.vector.tensor_scalar(
                        out=wsb[:], in0=ex[:], scalar1=rcp[:], scalar2=None,
                        op0=ALU.mult)
                    nc.sync.dma_start(out_attn.ap()[b:b + 1, :], wsb[:])

                    # w -> wT (bf16, two partition chunks: 128 + 68)
                    wbf = smp.tile([1, P], BF16, tag="wbf")
                    nc.vector.tensor_copy(wbf[:], wsb[:])
                    wt = smp.tile([128, 2], BF16, tag="wt")
                    ps = tp_ps.tile([128, 128], BF16, tag="tp")
                    nc.tensor.transpose(ps[:, :1], wbf[:, :128], identb[:1, :1])
                    nc.vector.tensor_copy(wt[:, 0:1], ps[:, :1])
                    ps = tp_ps.tile([128, 128], BF16, tag="tp")
                    nc.tensor.transpose(ps[:P - 128, :1], wbf[:, 128:P],
                                        identb[:1, :1])
                    nc.vector.tensor_copy(wt[:P - 128, 1:2], ps[:P - 128, :1])

                    # context = w @ enc_b  (re-DMA bf16 rows from stage)
                    eb0 = ebp.tile([128, E], BF16, tag="eb")
                    eb1 = ebp.tile([128, E], BF16, tag="eb")
                    nc.sync.dma_start(eb0[:], stage[b * P: b * P + 128, :])
                    nc.sync.dma_start(eb1[:P - 128, :],
                                      stage[b * P + 128:(b + 1) * P, :])
                    ctx_sb = ctxp.tile([1, E], F32, tag="ctx")
                    for ec in range(4):
                        cps = ctxps.tile([1, 512], F32, tag="cps")
                        nc.tensor.matmul(
                            cps[:], wt[:, 0:1],
                            eb0[:, ec * 512:(ec + 1) * 512],
                            start=True, stop=False)
                        nc.tensor.matmul(
                            cps[:], wt[:P - 128, 1:2],
                            eb1[:P - 128, ec * 512:(ec + 1) * 512],
                            start=False, stop=True)
                        nc.any.tensor_copy(ctx_sb[:, ec * 512:(ec + 1) * 512],
                                           cps[:])
                    nc.sync.dma_start(out_ctx.ap()[b:b + 1, :], ctx_sb[:])

    nc.compile()
    return nc


_NC = None


def _get_nc():
    global _NC
    if _NC is None:
        _NC = build_nc()
    return _NC


def kernel(enc_hiddens, dec_prev_hidden, W1_w, W1_b, W2_w, W2_b, V_w, V_b):
    from concourse import bass_utils

    nc = _get_nc()
    enc_hiddens = np.asarray(enc_hiddens, np.float32)
    dec_prev_hidden = np.asarray(dec_prev_hidden, np.float32)
    shared = {
        "W1_w": np.ascontiguousarray(W1_w, np.float32),
        "W1_b": np.ascontiguousarray(W1_b, np.float32),
        "W2_w": np.ascontiguousarray(W2_w, np.float32),
        "W2_b": np.ascontiguousarray(W2_b, np.float32),
        "V_w": np.ascontiguousarray(V_w, np.float32),
    }
    in_maps = []
    for i in range(NCORES):
        m = dict(shared)
        m["enc_hiddens"] = np.ascontiguousarray(enc_hiddens[i * BL:(i + 1) * BL])
        m["dec_prev_hidden"] = np.ascontiguousarray(
            dec_prev_hidden[i * BL:(i + 1) * BL])
        in_maps.append(m)

    res = bass_utils.run_bass_kernel_spmd(nc, in_maps, core_ids=list(range(NCORES)))
    outs = res.results
    context = np.concatenate([o["out_ctx"] for o in outs], axis=0).reshape(B, 1, E)
    attn = np.concatenate([o["out_attn"] for o in outs], axis=0)
    return context, attn


# revision 17
# speedup vs baseline: 2.2394x; 1.1643x over previous
"""Bahdanau-attention kernel for Trainium2, data-parallel over batch on 8 NeuronCores.

Per-core shard: 8 batches. Pipeline per core:
  1. Setup: cast-DMA W2/W1/dec to bf16, PE-transpose to get E on partitions,
     h_proj = dec @ W1.T in f32 PSUM, fold W1_b+W2_b into per-partition bias
     columns Hb.
  2. Main loop over flat (b*p) row chunks [384,384,384,416]:
     cast-DMA enc rows (f32->bf16) -> PE-transpose to encT -> bf16 matmuls
     accumulate enc_projT in f32 PSUM -> ScalarE tanh with per-partition bias
     Hb -> V-dot matmul accumulates scores.
  3. Per completed batch: softmax (f32) on its scores row, attn out, w -> wT
     transpose, context matmul (wT stationary, re-DMA'd bf16 enc rows moving).
Matmul operands are bf16 (1 cyc/row on PE); all accumulation is f32.
"""

import sys

sys.path.insert(0, "/opt/trn_rl_repo")
sys.path.insert(0, "/opt/pypackages")

import numpy as np
from contextlib import ExitStack

import concourse.bass as bass
import concourse.bacc as bacc
import concourse.mybir as mybir
import concourse.tile as tile
from concourse.masks import make_identity

F32 = mybir.dt.float32
BF16 = mybir.dt.bfloat16
AF = mybir.ActivationFunctionType
ALU = mybir.AluOpType
AX = mybir.AxisListType

B, P, E, A = 64, 196, 2048, 1024
NCORES = 8
BL = B // NCORES          # 8 local batches per core
BP = BL * P               # 1568 flat rows per core
KE = E // 128             # 16 contraction chunks
KA = A // 128             # 8 attn-dim tiles
CHUNKS = [(0, 384), (384, 384), (768, 384), (1152, 416)]
CW = 416                  # max chunk width


def _segments(S, W):
    """Batch segments [(b, off_in_chunk, len)] covering chunk rows [S, S+W)."""
    segs = []
    b = S // P
    pos = S
    while pos < S + W:
        end = min((b + 1) * P, S + W)
        segs.append((b, pos - S, end - pos))
        pos = end
        b += 1
    return segs


def _done_batches():
    done = []
    prev = 0
    for S, W in CHUNKS:
        nb = (S + W) // P
        done.append(list(range(prev, nb)))
        prev = nb
    return done


DONE = _done_batches()


def build_nc():
    nc = bacc.Bacc("TRN2", target_bir_lowering=False, debug=False,
                   enable_asserts=False)

    enc = nc.dram_tensor("enc_hiddens", [BL, P, E], F32, kind="ExternalInput")
    dec = nc.dram_tensor("dec_prev_hidden", [BL, E], F32, kind="ExternalInput")
    w1 = nc.dram_tensor("W1_w", [A, E], F32, kind="ExternalInput")
    w1b = nc.dram_tensor("W1_b", [A], F32, kind="ExternalInput")
    w2 = nc.dram_tensor("W2_w", [A, E], F32, kind="ExternalInput")
    w2b = nc.dram_tensor("W2_b", [A], F32, kind="ExternalInput")
    vw = nc.dram_tensor("V_w", [1, A], F32, kind="ExternalInput")
    out_ctx = nc.dram_tensor("out_ctx", [BL, E], F32, kind="ExternalOutput")
    out_attn = nc.dram_tensor("out_attn", [BL, P], F32, kind="ExternalOutput")

    enc_flat = enc.ap().rearrange("b p e -> (b p) e")

    with tile.TileContext(nc) as tc:
        with ExitStack() as ctx:
            const = ctx.enter_context(tc.tile_pool(name="const", bufs=1))
            identb = const.tile([128, 128], BF16)
            make_identity(nc, identb[:])
            identf = const.tile([128, 128], F32)
            make_identity(nc, identf[:])

            # Persistent SBUF tensors
            w2t = const.tile([128, KE * A], BF16)       # W2^T: [e, k-major a]
            hb = const.tile([128, KA * BL], F32)        # Hb[q, a*BL+b]
            vt = const.tile([128, KA], BF16)            # V^T columns per a-tile
            scores = const.tile([1, BP], F32)

            tp_ps = ctx.enter_context(
                tc.tile_pool(name="tp_ps", bufs=2, space="PSUM"))

            dramp = ctx.enter_context(tc.tile_pool(name="dram", bufs=1, space="DRAM"))
            # separate DRAM tiles so transpose-reads only depend on their own
            # cast (Tile tracks DRAM deps per tile, not per range)
            stages = [dramp.tile([W, E], BF16, tag=f"st{ci}", name=f"stage{ci}")
                      for ci, (S, W) in enumerate(CHUNKS)]
            w2stage = [dramp.tile([A, 512], BF16, tag=f"w2s{q}", name=f"w2stage{q}") for q in range(4)]
            w1stage = [dramp.tile([A, 512], BF16, tag=f"w1s{q}", name=f"w1stage{q}") for q in range(4)]

            # Cast-DMA order: enc chunks 0-1 first (unblock first matmuls),
            # then weights (sliced so transposed reads start early), then the
            # remaining enc chunks.
            def cast_chunk(ci):
                S, W = CHUNKS[ci]
                nc.gpsimd.dma_start(stages[ci][:], enc_flat[S:S + W, :])

            cast_chunk(0)
            cast_chunk(1)
            for q in range(4):
                nc.gpsimd.dma_start(w2stage[q][:],
                                    w2.ap()[:, q * 512:(q + 1) * 512])
            for q in range(4):
                nc.gpsimd.dma_start(w1stage[q][:],
                                    w1.ap()[:, q * 512:(q + 1) * 512])
            cast_chunk(2)
            cast_chunk(3)

            # ---------------- setup ----------------
            with ExitStack() as sctx:
                sp1 = sctx.enter_context(tc.tile_pool(name="setup1", bufs=1))
                hps_pool = sctx.enter_context(
                    tc.tile_pool(name="hps", bufs=2, space="PSUM"))

                # W2T / W1T via DRAM->SBUF xbar DMA transpose. One instruction
                # per 512-col cast slice, 3D destination [128, 4k, A].
                w2t3 = w2t[:].rearrange("p (k a) -> p k a", k=KE)
                w1t = sp1.tile([128, KE * A], BF16)
                w1t3 = w1t[:].rearrange("p (k a) -> p k a", k=KE)
                for q in range(4):
                    nc.scalar.dma_start_transpose(
                        w2t3[:, 4 * q: 4 * (q + 1), :], w2stage[q][:])
                for q in range(4):
                    nc.scalar.dma_start_transpose(
                        w1t3[:, 4 * q: 4 * (q + 1), :], w1stage[q][:])

                # dec -> decT (bf16)
                dec_sb = sp1.tile([BL, E], BF16)
                nc.gpsimd.dma_start(dec_sb[:], dec.ap())
                dect = sp1.tile([128, KE * BL], BF16)
                for k in range(KE):
                    ps = tp_ps.tile([128, 128], BF16, tag="tp")
                    nc.tensor.transpose(
                        ps[:, :BL], dec_sb[:, k * 128:(k + 1) * 128],
                        identb[:BL, :BL])
                    nc.any.tensor_copy(dect[:, k * BL:(k + 1) * BL], ps[:, :BL])

                # h_proj = dec @ W1.T  -> [BL, A] f32
                h_sb = sp1.tile([BL, A], F32)
                for half in range(2):
                    hps = hps_pool.tile([BL, 512], F32, tag="hps")
                    for k in range(KE):
                        nc.tensor.matmul(
                            hps[:],
                            dect[:, k * BL:(k + 1) * BL],
                            w1t[:, k * A + half * 512: k * A + half * 512 + 512],
                            start=(k == 0), stop=(k == KE - 1))
                    nc.any.tensor_copy(h_sb[:, half * 512:(half + 1) * 512], hps[:])

                # bias columns: W1_b + W2_b, laid out [q, a]  (f32)
                w1bc = sp1.tile([128, KA], F32)
                w2bc = sp1.tile([128, KA], F32)
                with nc.allow_non_contiguous_dma(reason="tiny bias transpose loads"):
                    nc.sync.dma_start(w1bc[:], w1b.ap().rearrange("(a k) -> k a", k=128))
                    nc.sync.dma_start(w2bc[:], w2b.ap().rearrange("(a k) -> k a", k=128))
                    nc.gpsimd.dma_start(vt[:], vw.ap().rearrange("o (a k) -> k (o a)", k=128))
                nc.vector.tensor_add(w1bc[:], w1bc[:], w2bc[:])

                # hT + bias -> Hb  (f32 transpose of h_sb)
                for a in range(KA):
                    ps = hps_pool.tile([128, 128], F32, tag="tpf")
                    nc.tensor.transpose(
                        ps[:, :BL], h_sb[:, a * 128:(a + 1) * 128],
                        identf[:BL, :BL])
                    nc.vector.tensor_scalar(
                        out=hb[:, a * BL:(a + 1) * BL], in0=ps[:, :BL],
                        scalar1=w1bc[:, a:a + 1], scalar2=None, op0=ALU.add)

            # ---------------- main loop ----------------
            enctp = ctx.enter_context(tc.tile_pool(name="enct", bufs=2))
            tpool = ctx.enter_context(tc.tile_pool(name="tpool", bufs=3))
            ebp = ctx.enter_context(tc.tile_pool(name="eb", bufs=4))
            ctxp = ctx.enter_context(tc.tile_pool(name="ctxsb", bufs=2))
            smp = ctx.enter_context(tc.tile_pool(name="smp", bufs=2))
            bigps = ctx.enter_context(tc.tile_pool(name="bigps", bufs=3, space="PSUM"))
            scps = ctx.enter_context(tc.tile_pool(name="scps", bufs=1, space="PSUM"))
            ctxps = ctx.enter_context(tc.tile_pool(name="ctxps", bufs=1, space="PSUM"))

            for ci, (S, W) in enumerate(CHUNKS):
                enct = enctp.tile([128, KE * CW], BF16, tag="enct")
                enct3 = enct[:].rearrange("p (k w) -> p k w", w=CW)
                nc.sync.dma_start_transpose(enct3[:, :, :W], stages[ci][:])

                segs = _segments(S, W)
                sc = scps.tile([1, CW], F32, tag="sc")
                for a in range(KA):
                    ps = bigps.tile([128, CW], F32, tag="big")
                    for k in range(KE):
                        nc.tensor.matmul(
                            ps[:, :W],
                            w2t[:, k * A + a * 128: k * A + (a + 1) * 128],
                            enct[:, k * CW: k * CW + W],
                            start=(k == 0), stop=(k == KE - 1))
                    t_sb = tpool.tile([128, CW], BF16, tag="t")
                    for (b, off, ln) in segs:
                        nc.scalar.activation(
                            t_sb[:, off:off + ln], ps[:, off:off + ln], AF.Tanh,
                            bias=hb[:, a * BL + b: a * BL + b + 1])
                    nc.tensor.matmul(
                        sc[:, :W], vt[:, a:a + 1], t_sb[:, :W],
                        start=(a == 0), stop=(a == KA - 1))
                nc.any.tensor_copy(scores[:, S:S + W], sc[:, :W])

                for b in DONE[ci]:
                    sseg = scores[:, b * P:(b + 1) * P]
                    negmax = smp.tile([1, 1], F32, tag="nm")
                    nc.vector.reduce_max(negmax[:], sseg, axis=AX.X, negate=True)
                    ex = smp.tile([1, P], F32, tag="ex")
                    nc.scalar.activation(ex[:], sseg, AF.Exp, bias=negmax[:])
                    ssum = smp.tile([1, 1], F32, tag="sm")
                    nc.vector.reduce_sum(ssum[:], ex[:], axis=AX.X)
                    rcp = smp.tile([1, 1], F32, tag="rc")
                    nc.vector.reciprocal(rcp[:], ssum[:])
                    wsb = smp.tile([1, P], F32, tag="w")
                    nc.vector.tensor_scalar(
                        out=wsb[:], in0=ex[:], scalar1=rcp[:], scalar2=None,
                        op0=ALU.mult)
                    nc.sync.dma_start(out_attn.ap()[b:b + 1, :], wsb[:])

                    # w -> wT (bf16, two partition chunks: 128 + 68)
                    wbf = smp.tile([1, P], BF16, tag="wbf")
                    nc.vector.tensor_copy(wbf[:], wsb[:])
                    wt = smp.tile([128, 2], BF16, tag="wt")
                    ps = tp_ps.tile([128, 128], BF16, tag="tp")
                    nc.tensor.transpose(ps[:, :1], wbf[:, :128], identb[:1, :1])
                    nc.vector.tensor_copy(wt[:, 0:1], ps[:, :1])
                    ps = tp_ps.tile([128, 128], BF16, tag="tp")
                    nc.tensor.transpose(ps[:P - 128, :1], wbf[:, 128:P],
                                        identb[:1, :1])
                    nc.vector.tensor_copy(wt[:P - 128, 1:2], ps[:P - 128, :1])

                    # context = w @ enc_b  (re-DMA rows as bf16 from f32 src)
                    eb0 = ebp.tile([128, E], BF16, tag="eb")
                    eb1 = ebp.tile([128, E], BF16, tag="eb")
                    nc.gpsimd.dma_start(eb0[:], enc_flat[b * P: b * P + 128, :])
                    nc.gpsimd.dma_start(eb1[:P - 128, :],
                                        enc_flat[b * P + 128:(b + 1) * P, :])
                    ctx_sb = ctxp.tile([1, E], F32, tag="ctx")
                    for ec in range(4):
                        cps = ctxps.tile([1, 512], F32, tag="cps")
                        nc.tensor.matmul(
                            cps[:], wt[:, 0:1],
                            eb0[:, ec * 512:(ec + 1) * 512],
                            start=True, stop=False)
                        nc.tensor.matmul(
                            cps[:], wt[:P - 128, 1:2],
                            eb1[:P - 128, ec * 512:(ec + 1) * 512],
                            start=False, stop=True)
                        nc.any.tensor_copy(ctx_sb[:, ec * 512:(ec + 1) * 512],
                                           cps[:])
                    nc.sync.dma_start(out_ctx.ap()[b:b + 1, :], ctx_sb[:])

    nc.compile()
    return nc


_NC = None


def _get_nc():
    global _NC
    if _NC is None:
        _NC = build_nc()
    return _NC


def kernel(enc_hiddens, dec_prev_hidden, W1_w, W1_b, W2_w, W2_b, V_w, V_b):
    from concourse import bass_utils

    nc = _get_nc()
    enc_hiddens = np.asarray(enc_hiddens, np.float32)
    dec_prev_hidden = np.asarray(dec_prev_hidden, np.float32)
    shared = {
        "W1_w": np.ascontiguousarray(W1_w, np.float32),
        "W1_b": np.ascontiguousarray(W1_b, np.float32),
        "W2_w": np.ascontiguousarray(W2_w, np.float32),
        "W2_b": np.ascontiguousarray(W2_b, np.float32),
        "V_w": np.ascontiguousarray(V_w, np.float32),
    }
    in_maps = []
    for i in range(NCORES):
        m = dict(shared)
        m["enc_hiddens"] = np.ascontiguousarray(enc_hiddens[i * BL:(i + 1) * BL])
        m["dec_prev_hidden"] = np.ascontiguousarray(
            dec_prev_hidden[i * BL:(i + 1) * BL])
        in_maps.append(m)

    res = bass_utils.run_bass_kernel_spmd(nc, in_maps, core_ids=list(range(NCORES)))
    outs = res.results
    context = np.concatenate([o["out_ctx"] for o in outs], axis=0).reshape(B, 1, E)
    attn = np.concatenate([o["out_attn"] for o in outs], axis=0)
    return context, attn
